# revision 10
# baseline (speedup 1.0000x reference)
"""Baseline v2 kernel (reconstructed) — device health check."""

import numpy as np

import concourse.bass as bass
import concourse.mybir as mybir
import concourse.tile as tile
from concourse import bacc
from concourse.bass import ts
from concourse.bass_utils import run_bass_kernel_spmd
from concourse.masks import make_identity

N_CORES = 8
B = 8192
BL = B // N_CORES          # 1024 batch rows per core
KF = 1024                  # IN_F (contraction)
NF = 4096                  # OUT_F
WL = NF // N_CORES         # 512 W rows per core
M = 1024                   # pooled features (32*32)
TOT = float(B * M)         # elements in the global mean
F32 = mybir.dt.float32
BF16 = mybir.dt.bfloat16
ADD = mybir.AluOpType.add
MULT = mybir.AluOpType.mult

_CACHE = {}


def build_nc():
    nc = bacc.Bacc("TRN2", target_bir_lowering=False, debug=False,
                   num_devices=N_CORES)
    x = nc.dram_tensor("x", [BL, KF], F32, kind="ExternalInput").ap()
    y = nc.dram_tensor("y", [BL, NF], F32, kind="ExternalInput").ap()
    w = nc.dram_tensor("w", [WL, KF], F32, kind="ExternalInput").ap()
    b = nc.dram_tensor("b", [1, NF], F32, kind="ExternalInput").ap()
    out = nc.dram_tensor("out", [BL, M], F32, kind="ExternalOutput").ap()

    w_pairs = w.rearrange("(n s) k -> n (s k)", s=2)          # [256, 2048]
    wv = w_pairs.rearrange("(a r j) kk -> r j a kk", a=4, r=2, j=32)

    with tile.TileContext(nc) as tc:
        with (
            tc.tile_pool(name="consts", bufs=1) as consts,
            tc.tile_pool(name="wload", bufs=1) as wload,
            tc.tile_pool(name="wtmp", bufs=1) as wtmp,
            tc.tile_pool(name="wtp", bufs=1) as wtp,
            tc.tile_pool(name="xload", bufs=8) as xload,
            tc.tile_pool(name="xtp", bufs=1) as xtp,
            tc.tile_pool(name="yload", bufs=4) as yload,
            tc.tile_pool(name="yup", bufs=3) as yup,
            tc.tile_pool(name="ysump", bufs=1) as ysump,
            tc.tile_pool(name="statsp", bufs=1) as statsp,
            tc.tile_pool(name="outp", bufs=3) as outp,
            tc.tile_pool(name="psA", bufs=4, space="PSUM") as psA,
            tc.tile_pool(name="psT", bufs=3, space="PSUM") as psT,
            tc.tile_pool(name="dram", bufs=1, space="DRAM") as dram,
        ):
            # ---- constants ----
            ident_f = consts.tile([128, 128], F32)
            make_identity(nc, ident_f)
            ident_b = consts.tile([128, 128], BF16)
            make_identity(nc, ident_b)
            ones_row = consts.tile([1, 128], BF16)
            nc.vector.memset(ones_row, 1.0)

            wl = wload.tile([128, 2, 2048], F32, tag="wl", name="wl")
            nc.scalar.dma_start(out=wl[:, 0, :], in_=wv[0])
            nc.sync.dma_start(out=wl[:, 1, :], in_=wv[1])
            wlv = wl.rearrange("p r (s k) -> p r s k", s=2)

            cc_w_in = dram.tile([128, KF], BF16, space="DRAM")
            cc_w_out = dram.tile([N_CORES * 128, KF], BF16, space="DRAM")
            wt8 = wtp.tile([128, 8, 8, 128], BF16)

            t1 = wtmp.tile([128, KF], F32)
            t2 = wtmp.tile([128, KF], F32)
            wsum = wtmp.tile([128, KF], BF16)
            wtc = wtmp.tile([128, 8, 128], BF16)
            bload = wload.tile([1, NF], F32, tag="wl", name="bload")
            bsum = consts.tile([1, 32, 32], F32)
            bsum_bf = consts.tile([1, M], BF16)
            btot = consts.tile([1, 1], F32)
            btot_s = consts.tile([1, 1], F32)

            xfs = []
            for bt in range(8):
                xf = xload.tile([128, KF], F32, tag="xf", name=f"xf{bt}")
                nc.sync.dma_start(out=xf, in_=x[ts(bt, 128), :])
                xfs.append(xf)
            xT_tiles = {}

            nc.vector.tensor_add(t1, wlv[:, 0, 0], wlv[:, 0, 1])
            nc.vector.tensor_add(t2, wlv[:, 1, 0], wlv[:, 1, 1])
            nc.vector.tensor_add(wsum, t1, t2)
            for kb in range(8):
                ptw = psT.tile([128, 128], BF16, tag="pt", name=f"ptw{kb}")
                nc.tensor.transpose(ptw, wsum[:, ts(kb, 128)], ident_b)
                nc.vector.tensor_copy(
                    out=wtc[:, kb, :].rearrange("k (a j) -> k j a", a=4),
                    in_=ptw.rearrange("k (j a) -> k j a", a=4))
            nc.gpsimd.dma_start(out=cc_w_in, in_=wtc)
            nc.gpsimd.collective_compute(
                "AllGather", mybir.AluOpType.bypass,
                replica_groups=[list(range(N_CORES))],
                ins=[cc_w_in.opt()], outs=[cc_w_out.opt()])

            stats = statsp.tile([128, 24], F32)
            xsum_acc = statsp.tile([128, 8], F32)
            wcol8 = statsp.tile([128, 8], F32)

            def xsred(k):
                xs_r = statsp.tile([128, 8, 1], F32, tag="xs_r", bufs=2,
                                   name=f"xs_r{k}")
                nc.vector.reduce_sum(out=xs_r, in_=xT_tiles[k],
                                     axis=mybir.AxisListType.X)
                if k == 0:
                    nc.vector.tensor_copy(out=xsum_acc, in_=xs_r[:, :, 0])
                else:
                    nc.vector.tensor_add(xsum_acc, xsum_acc, xs_r[:, :, 0])

            ys_tiles = {}
            for bt in range(8):
                ys = ysump.tile([128, M], F32, tag=f"ys{bt}", name=f"ys{bt}")
                for nh in range(2):
                    yt = yload.tile([128, 2048], F32, tag="yt",
                                    name=f"yt{bt}_{nh}")
                    nc.sync.dma_start(out=yt, in_=y[ts(bt, 128), ts(nh, 2048)])
                    ytv = yt.rearrange("p (q s) -> p q s", s=2)
                    u = yup.tile([128, KF], F32, tag="u", name=f"u{bt}_{nh}")
                    ueng = nc.vector if nh == 0 else nc.gpsimd
                    ueng.tensor_add(u, ytv[:, :, 0], ytv[:, :, 1])
                    u2 = u.rearrange("p (i r j) -> p i r j", r=2, j=32)
                    nc.vector.tensor_add(
                        ys[:, ts(nh, 512)].rearrange("p (i j) -> p i j", j=32),
                        u2[:, :, 0, :], u2[:, :, 1, :])
                    nc.vector.reduce_sum(
                        out=stats[:, 2 * bt + nh: 2 * bt + nh + 1],
                        in_=ys[:, ts(nh, 512)], axis=mybir.AxisListType.X)
                ys_tiles[bt] = ys

                xT = xtp.tile([128, 8, 128], BF16, tag=f"xT{bt}",
                              name=f"xT{bt}")
                for kb in range(8):
                    pt = psT.tile([128, 128], F32, tag="pt",
                                  name=f"ptx{bt}_{kb}")
                    nc.tensor.transpose(pt, xfs[bt][:, ts(kb, 128)], ident_f)
                    nc.scalar.copy(out=xT[:, kb, :], in_=pt)
                xT_tiles[bt] = xT

                if bt == 2:
                    nc.gpsimd.dma_start(out=bload, in_=b)
                if bt == 4:
                    blv = bload.rearrange("o (i r j s) -> o i r j s",
                                          r=2, j=32, s=2)
                    nc.vector.tensor_add(bsum, blv[:, :, 0, :, 0],
                                         blv[:, :, 0, :, 1])
                    nc.vector.tensor_add(bsum, bsum, blv[:, :, 1, :, 0])
                if bt == 5:
                    nc.vector.tensor_add(bsum, bsum, blv[:, :, 1, :, 1])
                    nc.vector.tensor_copy(
                        out=bsum_bf, in_=bsum.rearrange("o i j -> o (i j)"))
                if bt == 6:
                    nc.vector.reduce_sum(
                        out=btot, in_=bsum.rearrange("o i j -> o (i j)"),
                        axis=mybir.AxisListType.X)
                    nc.vector.tensor_scalar_mul(btot_s, btot, float(BL))
                if bt >= 3:
                    xsred(bt - 3)

            with tc.tile_wait_until(0.3):
                for r in range(8):
                    nc.scalar.dma_start(out=wt8[:, r],
                                        in_=cc_w_out[ts(r, 128), :])

            for k in (5, 6, 7):
                xsred(k)
            for kb in range(8):
                nc.vector.reduce_sum(out=wcol8[:, kb:kb + 1],
                                     in_=wt8[:, :, kb, :],
                                     axis=mybir.AxisListType.XY)
            nc.vector.tensor_mul(stats[:, 16:24], xsum_acc, wcol8)
            pcol = statsp.tile([128, 1], F32)
            nc.vector.reduce_sum(out=pcol, in_=stats,
                                 axis=mybir.AxisListType.X)
            nc.vector.tensor_add(pcol[0:1, :], pcol[0:1, :], btot_s)
            cc_ar_in = dram.tile([128, 1], F32, space="DRAM")
            cc_ar_out = dram.tile([128, 1], F32, space="DRAM")
            nc.sync.dma_start(out=cc_ar_in, in_=pcol)
            nc.gpsimd.collective_compute(
                "AllReduce", ADD,
                replica_groups=[list(range(N_CORES))],
                ins=[cc_ar_in.opt()], outs=[cc_ar_out.opt()])

            mmsb_tiles = {}
            for bt in range(8):
                mmsb = ysump.tile([128, M], BF16, tag=f"mb{bt}",
                                  name=f"mb{bt}")
                for mh in range(2):
                    mm = psA.tile([128, 512], F32, tag="mm",
                                  name=f"mm{bt}_{mh}")
                    for kb in range(8):
                        nc.tensor.matmul(mm, xT_tiles[bt][:, kb, :],
                                         wt8[:, 4 * mh:4 * mh + 4, kb, :],
                                         start=(kb == 0), stop=False)
                    nc.tensor.matmul(mm, ones_row, bsum_bf[:, ts(mh, 512)],
                                     start=False, stop=True)
                    nc.scalar.copy(out=mmsb[:, ts(mh, 512)], in_=mm)
                mmsb_tiles[bt] = mmsb

            with tc.tile_wait_until(0.5):
                for bt in range(8):
                    nc.vector.tensor_add(ys_tiles[bt], ys_tiles[bt],
                                         mmsb_tiles[bt])

            gsb = statsp.tile([128, 128], F32)
            nc.sync.dma_start(
                out=gsb,
                in_=cc_ar_out.rearrange("p o -> o p").to_broadcast((128, 128)))
            g1 = statsp.tile([128, 1], F32)
            nc.vector.reduce_sum(out=g1, in_=gsb, axis=mybir.AxisListType.X)
            rsb = statsp.tile([128, 1], F32)
            nc.vector.reciprocal(rsb, g1)

            for bt in range(8):
                meng = nc.vector if bt % 2 == 0 else nc.gpsimd
                ot = outp.tile([128, M], F32, tag="ot", name=f"ot{bt}")
                meng.tensor_scalar(out=ot, in0=ys_tiles[bt],
                                   scalar1=rsb, scalar2=TOT,
                                   op0=MULT, op1=MULT)
                reng = nc.sync if bt % 2 == 0 else nc.scalar
                reng.dma_start(out=out[ts(bt, 128), :], in_=ot)

    nc.compile()
    return nc


def _run(inputs, trace=False):
    if "nc" not in _CACHE:
        _CACHE["nc"] = build_nc()
    nc = _CACHE["nc"]
    x = np.ascontiguousarray(np.asarray(inputs["x"], dtype=np.float32))
    y = np.ascontiguousarray(np.asarray(inputs["y"], dtype=np.float32))
    w = np.ascontiguousarray(np.asarray(inputs["weight"], dtype=np.float32))
    b = np.ascontiguousarray(
        np.asarray(inputs["bias"], dtype=np.float32).reshape(1, NF))
    in_maps = [
        {"x": x[c * BL:(c + 1) * BL], "y": y[c * BL:(c + 1) * BL],
         "w": np.ascontiguousarray(w[c * WL:(c + 1) * WL]), "b": b}
        for c in range(N_CORES)
    ]
    res = run_bass_kernel_spmd(nc, in_maps, core_ids=list(range(N_CORES)),
                               trace=trace)
    full = np.concatenate([res.results[c]["out"] for c in range(N_CORES)],
                          axis=0)
    return full.reshape(B, 1, 32, 32), res


def kernel(**inputs) -> np.ndarray:
    out, _ = _run(inputs, trace=False)
    return out


# revision 12
# speedup vs baseline: 1.0032x; 1.0032x over previous
"""Fused GEMM + bias + residual + AvgPool2d(2) + global-mean normalize, 8-core SPMD.

Reference computation (B=8192, IN_F=1024, OUT_F=4096, S=64, K=2):
    out_lin = x @ W.T + bias + y                  # (B, 4096)
    pooled  = avgpool2x2(out_lin.reshape(B,64,64))# (B, 32, 32)
    out     = pooled / pooled.mean()              # (B, 1, 32, 32)

Key algebraic folds (all exact):
  * The 2x2 avg-pool is linear, so it folds into the weight/bias/residual:
        pooled_raw[b, m] = x[b] . Wsum[m] + bias_sum[m] + y_sum[b, m]
    where m = 32*i + j pools OUT_F rows {128i+2j, 128i+2j+1, 128i+64+2j,
    128i+64+2j+1}.  GEMM N-dim shrinks 4096 -> 1024.
  * The 1/4 pool factor cancels: out = pooled_raw * (B*1024 / gsum).
  * gsum = xsum_g . wcol_g + B*bias_tot + ytot_g.  The AllReduce payload
    carries the xsum and wcol VECTORS ([128,8] each) alongside the y-rowsum
    stats, and every core computes the dot AFTER the AllReduce -- so the
    AllReduce depends only on local data (never on the AllGather).

Distribution: batch split 8 ways; W sharded by row-block (core c loads rows
[512c, 512c+512), 2 MiB), pools to its 128 features, AllGathers the 256 KB
bf16 pooled shards (k-major, so the PE does no W transposes post-gather).

Schedule (v3) -- the big lesson from the v2 trace: the AllGather's wire
phase contended with the y-stream on the shared SDMA engines and took
~88us trigger-to-done (intrinsic cost ~10us).  v3 gives it a clear window:

  * sync (SP) ring:   wl half, y0, y1, [rb0-3: waits AG sem -- blocks the
    ring, creating the DMA-quiet window], y2..y7, AR payload, even stores.
  * scalar (ACT) ring: wl half, x0-3, xT0-3 drains (compute, no DMA -- the
    window stays quiet), rb4-7 (AG wait), x4-7, psum drains + xT4-7 drains,
    AR readback, odd stores.
  * The GEMM starts right after the readbacks (~35us) and overlaps the
    back half of the y-stream; psum drains feed ys += mm adds slotted
    mid-loop on the DVE queue.
  * AllReduce payload [128,32]: cols 0-15 y-pool rowsums (fused into the
    pool via tensor_tensor_reduce accumulators), 16-23 xsum, 24-31 wcol
    (from the local pooled-W transpose, pre-AllGather).  Trigger fires at
    y-stream end, fully independent of AllGather/GEMM.
  * Post-AR: dot + partition_all_reduce + reciprocal, then per-tile
    normalize + store as each tile's GEMM add lands (pipelined tail).
"""

import numpy as np

import concourse.bass as bass
import concourse.mybir as mybir
import concourse.tile as tile
from concourse import bacc
from concourse import bass_isa
from concourse.bass import ts
from concourse.bass_utils import run_bass_kernel_spmd
from concourse.masks import make_identity

N_CORES = 8
B = 8192
BL = B // N_CORES          # 1024 batch rows per core
KF = 1024                  # IN_F (contraction)
NF = 4096                  # OUT_F
WL = NF // N_CORES         # 512 W rows per core
M = 1024                   # pooled features (32*32)
TOT = float(B * M)         # elements in the global mean
F32 = mybir.dt.float32
BF16 = mybir.dt.bfloat16
ADD = mybir.AluOpType.add
MULT = mybir.AluOpType.mult

_CACHE = {}


def build_nc():
    nc = bacc.Bacc("TRN2", target_bir_lowering=False, debug=False,
                   num_devices=N_CORES)
    x = nc.dram_tensor("x", [BL, KF], F32, kind="ExternalInput").ap()
    y = nc.dram_tensor("y", [BL, NF], F32, kind="ExternalInput").ap()
    w = nc.dram_tensor("w", [WL, KF], F32, kind="ExternalInput").ap()
    b = nc.dram_tensor("b", [1, NF], F32, kind="ExternalInput").ap()
    out = nc.dram_tensor("out", [BL, M], F32, kind="ExternalOutput").ap()

    # This core's W rows n = 128a + 64r + 2j + s pool to local feature
    # m_local = 32a + j; (r, s) are the pool taps.  j-major load keeps DMA
    # descriptors wide; partition p = 4j + a.
    w_pairs = w.rearrange("(n s) k -> n (s k)", s=2)          # [256, 2048]
    wv = w_pairs.rearrange("(a r j) kk -> r j a kk", a=4, r=2, j=32)

    with tile.TileContext(nc) as tc:
        with (
            tc.tile_pool(name="consts", bufs=1) as consts,
            tc.tile_pool(name="wload", bufs=1) as wload,
            tc.tile_pool(name="wtmp", bufs=1) as wtmp,
            tc.tile_pool(name="wtp", bufs=1) as wtp,
            tc.tile_pool(name="xload", bufs=8) as xload,
            tc.tile_pool(name="xtp", bufs=1) as xtp,
            tc.tile_pool(name="yload", bufs=4) as yload,
            tc.tile_pool(name="yup", bufs=3) as yup,
            tc.tile_pool(name="ysump", bufs=1) as ysump,
            tc.tile_pool(name="statsp", bufs=1) as statsp,
            tc.tile_pool(name="outp", bufs=3) as outp,
            tc.tile_pool(name="psA", bufs=4, space="PSUM") as psA,
            tc.tile_pool(name="psT", bufs=3, space="PSUM") as psT,
            tc.tile_pool(name="dram", bufs=1, space="DRAM") as dram,
        ):
            # ---- constants ----
            ident_f = consts.tile([128, 128], F32)
            make_identity(nc, ident_f)
            ident_b = consts.tile([128, 128], BF16)
            make_identity(nc, ident_b)
            ones_row = consts.tile([1, 128], BF16)
            nc.vector.memset(ones_row, 1.0)
            ones_col_f = consts.tile([128, 1], F32)
            nc.vector.memset(ones_col_f, 1.0)
            ones_row_f = consts.tile([1, 128], F32)
            nc.vector.memset(ones_row_f, 1.0)

            # ---- queue heads: W shard (split across both rings), x0-3 on
            # the scalar ring, y0-1 on the sync ring.  Everything else on
            # each ring sits behind an AllGather-gated readback, which
            # creates the DMA-quiet window the AG mesh needs. ----
            wl = wload.tile([128, 2, 2048], F32, tag="wl", name="wl")
            nc.scalar.dma_start(out=wl[:, 0, :], in_=wv[0])
            nc.sync.dma_start(out=wl[:, 1, :], in_=wv[1])
            wlv = wl.rearrange("p r (s k) -> p r s k", s=2)

            xfs = []
            for bt in range(8):
                xf = xload.tile([128, KF], F32, tag="xf", name=f"xf{bt}")
                if bt < 4:
                    nc.scalar.dma_start(out=xf, in_=x[ts(bt, 128), :])
                xfs.append(xf)

            yts = {}
            for bt in range(2):
                for nh in range(2):
                    yt = yload.tile([128, 2048], F32, tag="yt",
                                    name=f"yt{bt}_{nh}")
                    nc.sync.dma_start(out=yt, in_=y[ts(bt, 128), ts(nh, 2048)])
                    yts[(bt, nh)] = yt

            # ---- stats: the AllReduce payload.  cols 0-15 y-pool rowsums,
            # 16-23 xsum (per k), 24-31 wcol partial (per k). ----
            stats = statsp.tile([128, 32], F32)

            # ---- W pooling + k-major transpose + AllGather, ASAP ----
            t1 = wtmp.tile([128, KF], F32)
            t2 = wtmp.tile([128, KF], F32)
            wsum = wtmp.tile([128, KF], BF16)
            wtc = wtmp.tile([128, 8, 128], BF16)
            nc.vector.tensor_add(t1, wlv[:, 0, 0], wlv[:, 0, 1])
            nc.vector.tensor_add(t2, wlv[:, 1, 0], wlv[:, 1, 1])
            nc.vector.tensor_add(wsum, t1, t2)
            for kb in range(8):
                ptw = psT.tile([128, 128], BF16, tag="pt", name=f"ptw{kb}")
                nc.tensor.transpose(ptw, wsum[:, ts(kb, 128)], ident_b)
                # undo the j-major load permutation p=4j+a -> 32a+j
                nc.vector.tensor_copy(
                    out=wtc[:, kb, :].rearrange("k (a j) -> k j a", a=4),
                    in_=ptw.rearrange("k (j a) -> k j a", a=4))
            # local wcol partial: sum over this core's 128 pooled features
            nc.vector.reduce_sum(
                out=stats[:, 24:32].rearrange("p (n o) -> p n o", o=1),
                in_=wtc, axis=mybir.AxisListType.X)

            cc_w_in = dram.tile([128, KF], BF16, space="DRAM")
            cc_w_out = dram.tile([N_CORES * 128, KF], BF16, space="DRAM")
            nc.gpsimd.dma_start(out=cc_w_in, in_=wtc)
            nc.gpsimd.collective_compute(
                "AllGather", mybir.AluOpType.bypass,
                replica_groups=[list(range(N_CORES))],
                ins=[cc_w_in.opt()], outs=[cc_w_out.opt()])

            # bias load (SWDGE, reuses the W slot once pooled)
            bload = wload.tile([1, NF], F32, tag="wl", name="bload")
            nc.gpsimd.dma_start(out=bload, in_=b)

            # ---- helpers ----
            ys_tiles = {}
            mmsb_tiles = {}
            xT_tiles = {}

            def pool_tile(bt):
                ys = ys_tiles[bt]
                for nh in range(2):
                    yt = yts.pop((bt, nh))
                    ytv = yt.rearrange("p (q s) -> p q s", s=2)
                    u = yup.tile([128, KF], F32, tag="u", name=f"u{bt}_{nh}")
                    ueng = nc.vector if nh == 0 else nc.gpsimd
                    ueng.tensor_add(u, ytv[:, :, 0], ytv[:, :, 1])
                    u2 = u.rearrange("p (i r j) -> p i r j", r=2, j=32)
                    c = 2 * bt + nh
                    nc.vector.tensor_add(
                        ys[:, ts(nh, 512)].rearrange("p (i j) -> p i j", j=32),
                        u2[:, :, 0, :], u2[:, :, 1, :])
                    nc.vector.reduce_sum(
                        out=stats[:, c:c + 1],
                        in_=ys[:, ts(nh, 512)], axis=mybir.AxisListType.X)

            def load_y(bt):
                for nh in range(2):
                    yt = yload.tile([128, 2048], F32, tag="yt",
                                    name=f"yt{bt}_{nh}")
                    nc.sync.dma_start(out=yt, in_=y[ts(bt, 128), ts(nh, 2048)])
                    yts[(bt, nh)] = yt

            def xtrans(bt):
                xT = xtp.tile([128, 8, 128], BF16, tag=f"xT{bt}",
                              name=f"xT{bt}")
                for kb in range(8):
                    pt = psT.tile([128, 128], F32, tag="pt",
                                  name=f"ptx{bt}_{kb}")
                    nc.tensor.transpose(pt, xfs[bt][:, ts(kb, 128)], ident_f)
                    nc.scalar.copy(out=xT[:, kb, :], in_=pt)
                xT_tiles[bt] = xT

            def xsred(k):
                xs_r = statsp.tile([128, 8, 1], F32, tag="xs_r", bufs=2,
                                   name=f"xs_r{k}")
                nc.vector.reduce_sum(out=xs_r, in_=xT_tiles[k],
                                     axis=mybir.AxisListType.X)
                if k == 0:
                    nc.vector.tensor_copy(out=stats[:, 16:24],
                                          in_=xs_r[:, :, 0])
                else:
                    nc.vector.tensor_add(stats[:, 16:24], stats[:, 16:24],
                                         xs_r[:, :, 0])

            for bt in range(8):
                ys_tiles[bt] = ysump.tile([128, M], F32, tag=f"ys{bt}",
                                          name=f"ys{bt}")

            # ---- early DVE work inside the AG window: y0/y1 pools, bias
            # pooling; early PE work: x0-3 transposes (ACT drains them --
            # compute only, the DMA rings stay quiet) ----
            pool_tile(0)
            pool_tile(1)

            bsum = consts.tile([1, 32, 32], F32)
            bsum_bf = consts.tile([1, M], BF16)
            btot = consts.tile([1, 1], F32)
            btot_s = consts.tile([1, 1], F32)
            blv = bload.rearrange("o (i r j s) -> o i r j s", r=2, j=32, s=2)
            nc.vector.tensor_add(bsum, blv[:, :, 0, :, 0], blv[:, :, 0, :, 1])
            nc.vector.tensor_add(bsum, bsum, blv[:, :, 1, :, 0])
            nc.vector.tensor_add(bsum, bsum, blv[:, :, 1, :, 1])
            nc.vector.tensor_copy(
                out=bsum_bf, in_=bsum.rearrange("o i j -> o (i j)"))
            nc.vector.reduce_sum(
                out=btot, in_=bsum.rearrange("o i j -> o (i j)"),
                axis=mybir.AxisListType.X)
            nc.vector.tensor_scalar_mul(btot_s, btot, float(BL))

            for bt in range(4):
                xtrans(bt)

            # ---- AllGather readbacks.  rb0-3 sit on the sync ring right
            # after y1; rb4-7 on the scalar ring after the xT0-3 drains.
            # Their AG-sem wait deliberately head-of-line-blocks both rings:
            # that IS the quiet window.  The gathered W is split in two so
            # each GEMM m-half starts as soon as its 4 readbacks land. ----
            wt8a = wtp.tile([128, 4, 8, 128], BF16)
            wt8b = wtp.tile([128, 4, 8, 128], BF16)
            for r in range(4):
                nc.sync.dma_start(out=wt8a[:, r], in_=cc_w_out[ts(r, 128), :])
            for r in range(4, 8):
                nc.scalar.dma_start(out=wt8b[:, r - 4],
                                    in_=cc_w_out[ts(r, 128), :])
            for bt in range(4, 8):
                nc.scalar.dma_start(out=xfs[bt], in_=x[ts(bt, 128), :])

            # ---- main loop: remaining y stream + pooling + xsums, with the
            # GEMM + drains + ys+=mm adds slotted in as results land ----
            def gemm(bt):
                mmsb = ysump.tile([128, M], BF16, tag=f"mb{bt}",
                                  name=f"mb{bt}")
                for mh in range(2):
                    mm = psA.tile([128, 512], F32, tag="mm",
                                  name=f"mm{bt}_{mh}")
                    wt8 = wt8a if mh == 0 else wt8b
                    for kb in range(8):
                        nc.tensor.matmul(mm, xT_tiles[bt][:, kb, :],
                                         wt8[:, :, kb, :],
                                         start=(kb == 0), stop=False)
                    nc.tensor.matmul(mm, ones_row, bsum_bf[:, ts(mh, 512)],
                                     start=False, stop=True)
                    nc.scalar.copy(out=mmsb[:, ts(mh, 512)], in_=mm)
                mmsb_tiles[bt] = mmsb

            def mmadd(bt):
                nc.vector.tensor_add(ys_tiles[bt], ys_tiles[bt],
                                     mmsb_tiles[bt])

            load_y(2)
            xsred(0)
            gemm(0)
            load_y(3)
            pool_tile(2)
            xsred(1)
            gemm(1)
            xtrans(4)
            load_y(4)
            pool_tile(3)
            xsred(2)
            gemm(2)
            xtrans(5)
            load_y(5)
            pool_tile(4)
            xsred(3)
            mmadd(0)
            gemm(3)
            xtrans(6)
            load_y(6)
            pool_tile(5)
            xsred(4)
            mmadd(1)
            gemm(4)
            xtrans(7)
            load_y(7)
            pool_tile(6)
            xsred(5)
            mmadd(2)
            gemm(5)
            pool_tile(7)
            xsred(6)
            mmadd(3)
            gemm(6)
            xsred(7)
            gemm(7)

            # fold the bias moment into the payload (after all col-0 writes)
            nc.vector.tensor_add(stats[0:1, 0:1], stats[0:1, 0:1], btot_s)

            # ---- AllReduce: payload write rides the sync ring right after
            # the last y tile; the trigger depends only on local stats ----
            cc_ar_in = dram.tile([128, 32], F32, space="DRAM")
            cc_ar_out = dram.tile([128, 32], F32, space="DRAM")
            nc.sync.dma_start(out=cc_ar_in, in_=stats)
            nc.gpsimd.collective_compute(
                "AllReduce", ADD,
                replica_groups=[list(range(N_CORES))],
                ins=[cc_ar_in.opt()], outs=[cc_ar_out.opt()])

            # remaining GEMM adds (DVE, inside the AllReduce window)
            for bt in range(4, 8):
                mmadd(bt)

            # ---- post-AR: gsum = sum(stats cols 0-15) + xsum_g . wcol_g ----
            S = statsp.tile([128, 32], F32)
            nc.scalar.dma_start(out=S, in_=cc_ar_out)
            tmp8 = statsp.tile([128, 8], F32)
            nc.vector.tensor_mul(tmp8, S[:, 16:24], S[:, 24:32])
            red1 = statsp.tile([128, 1], F32)
            red2 = statsp.tile([128, 1], F32)
            nc.vector.reduce_sum(out=red1, in_=S[:, 0:16],
                                 axis=mybir.AxisListType.X)
            nc.vector.reduce_sum(out=red2, in_=tmp8,
                                 axis=mybir.AxisListType.X)
            pcol2 = statsp.tile([128, 1], F32)
            nc.vector.tensor_add(pcol2, red1, red2)
            # partition-sum + broadcast on the (idle) PE: colsum via ones
            # then rank-1 broadcast back to 128 partitions
            g1ps = psT.tile([128, 128], F32, tag="pt", name="g1ps")
            nc.tensor.matmul(g1ps[0:1, 0:1], ones_col_f, pcol2,
                             start=True, stop=True)
            g1sb = statsp.tile([1, 1], F32)
            nc.scalar.copy(out=g1sb, in_=g1ps[0:1, 0:1])
            gbps = psT.tile([128, 128], F32, tag="pt", name="gbps")
            nc.tensor.matmul(gbps[:, 0:1], ones_row_f, g1sb,
                             start=True, stop=True)
            rsb = statsp.tile([128, 1], F32)
            nc.vector.reciprocal(rsb, gbps[:, 0:1])

            # ---- normalize + store, per-tile pipelined ----
            for bt in range(8):
                meng = nc.vector if bt % 2 == 0 else nc.gpsimd
                ot = outp.tile([128, M], F32, tag="ot", name=f"ot{bt}")
                meng.tensor_scalar(out=ot, in0=ys_tiles[bt],
                                   scalar1=rsb, scalar2=TOT,
                                   op0=MULT, op1=MULT)
                reng = nc.sync if bt % 2 == 0 else nc.scalar
                reng.dma_start(out=out[ts(bt, 128), :], in_=ot)

    nc.compile()
    return nc


def _run(inputs, trace=False):
    if "nc" not in _CACHE:
        _CACHE["nc"] = build_nc()
    nc = _CACHE["nc"]
    x = np.ascontiguousarray(np.asarray(inputs["x"], dtype=np.float32))
    y = np.ascontiguousarray(np.asarray(inputs["y"], dtype=np.float32))
    w = np.ascontiguousarray(np.asarray(inputs["weight"], dtype=np.float32))
    b = np.ascontiguousarray(
        np.asarray(inputs["bias"], dtype=np.float32).reshape(1, NF))
    in_maps = [
        {"x": x[c * BL:(c + 1) * BL], "y": y[c * BL:(c + 1) * BL],
         "w": np.ascontiguousarray(w[c * WL:(c + 1) * WL]), "b": b}
        for c in range(N_CORES)
    ]
    res = run_bass_kernel_spmd(nc, in_maps, core_ids=list(range(N_CORES)),
                               trace=trace)
    full = np.concatenate([res.results[c]["out"] for c in range(N_CORES)],
                          axis=0)
    return full.reshape(B, 1, 32, 32), res


def kernel(**inputs) -> np.ndarray:
    out, _ = _run(inputs, trace=False)
    return out
